# revision 19
# baseline (speedup 1.0000x reference)
"""Causal self-attention Trainium2 kernel (B=8, T=2048, C=256, H=4).

Sharding: batch B=8 across the 8 NeuronCores (data parallel, no collectives).
Each core computes one batch element end-to-end:
  qkv = x @ W_attn ; per-head causal softmax(q k^T / sqrt(hs)) @ v ; @ W_proj

Layout strategy (per core):
  - x streamed in 4 token chunks of 512: per chunk DMA -> bf16 cast (DVE)
    -> PE transpose -> q/k matmuls (drained on ACT; Copy shares the exp
    act table) -> v matmuls (drained on DVE). Per-chunk tiles keep the
    dependencies fine-grained so the PE starts as soon as chunk 0 lands.
  - qT,kT computed transposed (feature rows on partitions); the softmax
    scale * log2(e) is folded into the W_attn q-columns at the bf16 cast
    so scores come out of the PE in log2 units.
  - S^T tiles (k on partitions, q on free dim) = kT_tile.T @ qT_block; the
    two heads of a pair are emitted back-to-back with K=64 row groups 0/64
    so they pack concurrently in the PE array (ATT_GROUP=2 keeps every
    pair emission-adjacent).
  - exp2 via ScalarE activation(Exp, scale=ln2) over 2-bank PSUM groups.
  - causal mask on diagonal 128x128 blocks via gpsimd affine_select on P.
  - O^T += V_tile.T @ P (per-head accumulators, M=65: the 65th stationary
    column is ones so row 64 of O^T accumulates the softmax row sums).
  - Software pipelining: emission order scores(g), scores(g+1), PV(g), ...
    so the in-order PE never waits on the ACT exp of the current group;
    oacc bufs=4 so a new round's PV never waits on the previous round's
    normalization reads.
  - normalization: O^T staged to SBUF (PSUM-direct reads measurably slow
    the PE/ACT streams via PSUM port contention), sums broadcast from
    partition 0, custom-DVE reciprocal_approx_fast, two multiplies into
    per-round yT tiles.
  - proj tail: z = Y @ W_proj, deep-buffered (psz bufs=4, z bufs=8) so the
    output DMA latency never stalls the matmul stream.
"""

import sys

if "/opt/trn_rl_repo" not in sys.path:
    sys.path.insert(0, "/opt/trn_rl_repo")

import numpy as np

import concourse.bass as bass
import concourse.mybir as mybir
from concourse import bacc
from concourse.masks import make_identity
from concourse.tile import TileContext

B, T, C = 8, 2048, 256
H, HS = 4, 64
NT = T // 128            # 16 token tiles
NQB = T // 512           # 4 q blocks of 512
F32 = mybir.dt.float32
BF16 = mybir.dt.bfloat16
LOG2E = 1.4426950408889634
LN2 = 0.6931471805599453
QSCALE = LOG2E / 8.0     # softmax scale 1/sqrt(hs) in log2 units
ATT_GROUP = 2            # S tiles per exp2 activation (one head pair)

_cached_nc = None


def _build():
    nc = bacc.Bacc("TRN2", target_bir_lowering=False, debug=False)
    x_d = nc.declare_dram_parameter("x", [T, C], F32, isOutput=False)
    wa_d = nc.declare_dram_parameter("W_attn", [C, 3 * C], F32, isOutput=False)
    wp_d = nc.declare_dram_parameter("W_proj", [C, C], F32, isOutput=False)
    y_d = nc.declare_dram_parameter("y", [T, C], F32, isOutput=True)

    with TileContext(nc) as tc:
        sb = tc.alloc_tile_pool(name="sb", bufs=1)
        # per-chunk input tiles (chunk = 512 tokens = 4 token tiles)
        x_c = [sb.tile([128, 1024], F32, name=f"x{c}") for c in range(4)]
        xb_c = [sb.tile([128, 1024], BF16, name=f"xb{c}") for c in range(4)]
        # xT chunk: [c_in 128, (kc, t512)]
        xT_c = [sb.tile([128, 1024], BF16, name=f"xT{c}") for c in range(4)]
        # qT/kT per (fh, nb): [feat 128, t512]; feat rows 0-63 = head 2fh,
        # 64-127 = head 2fh+1
        qTt = [[sb.tile([128, 512], BF16, name=f"qT{fh}_{nb}")
                for nb in range(NQB)] for fh in range(2)]
        kTt = [[sb.tile([128, 512], BF16, name=f"kT{fh}_{nb}")
                for nb in range(NQB)] for fh in range(2)]
        # v with a ones column per head: [t128, (n4, gh, 65)]; col 64 == 1.0
        v65c = [sb.tile([128, 4 * 260], BF16, name=f"v65_{c}")
                for c in range(4)]
        # yT split per (hp, tqb)
        yTt = [[sb.tile([128, 512], BF16, name=f"yT{hp}_{tqb}")
                for tqb in range(NQB)] for hp in range(2)]
        wa_f = sb.tile([128, 2 * 768], F32, name="wa_f")
        wa_b = sb.tile([128, 2 * 768], BF16, name="wa_b")
        wp_f = sb.tile([128, 2 * 256], F32, name="wp_f")
        wp_b = sb.tile([128, 2 * 256], BF16, name="wp_b")
        ident = sb.tile([128, 128], F32, name="ident")
        identb = sb.tile([128, 128], BF16, name="identb")

        make_identity(nc, ident)
        nc.vector.tensor_copy(identb, ident)
        for c in range(4):
            nc.gpsimd.memset(v65c[c], 1.0)  # ones cols survive the v copies

        # ---- load inputs (x in 4 chunks so the pipeline starts early) ----
        nc.sync.dma_start(
            wa_f.rearrange("p (k m) -> p k m", k=2),
            wa_d[:].rearrange("(k p) m -> p k m", p=128),
        )
        nc.sync.dma_start(
            wp_f.rearrange("p (k m) -> p k m", k=2),
            wp_d[:].rearrange("(k p) m -> p k m", p=128),
        )
        for c in range(4):
            nc.sync.dma_start(
                x_c[c].rearrange("p (n c2) -> p n c2", n=4),
                x_d[c * 512:(c + 1) * 512].rearrange(
                    "(n p) c2 -> p n c2", p=128),
            )
        # bf16 weight cast; QSCALE folded into the q columns of W_attn
        for kc in range(2):
            nc.vector.tensor_scalar_mul(
                wa_b[:, kc * 768: kc * 768 + 256],
                wa_f[:, kc * 768: kc * 768 + 256],
                QSCALE,
            )
            nc.vector.tensor_copy(
                wa_b[:, kc * 768 + 256: kc * 768 + 768],
                wa_f[:, kc * 768 + 256: kc * 768 + 768],
            )
        nc.vector.tensor_copy(wp_b[:], wp_f[:])

        # ---- streamed setup: per chunk cast -> transpose -> qkv ----
        with tc.tile_pool(name="pset", bufs=1, space="PSUM") as pset:
            for c in range(4):
                nc.vector.tensor_copy(xb_c[c], x_c[c])
                for kc in range(2):
                    tp = pset.tile([128, 512], BF16, tag="tp", bufs=2)
                    for j in range(4):
                        nc.tensor.transpose(
                            tp[:, j * 128:(j + 1) * 128],
                            xb_c[c][:, j * 256 + kc * 128:
                                    j * 256 + kc * 128 + 128],
                            identb,
                        )
                    nc.vector.tensor_copy(
                        xT_c[c][:, kc * 512:(kc + 1) * 512], tp[:]
                    )
                # q/k for this token block (nb == c), both feature halves
                for fh in range(2):
                    ps_q = pset.tile([128, 512], F32, tag="mm", bufs=2)
                    nc.tensor.matmul(
                        ps_q,
                        wa_b[:, 0 * 768 + fh * 128: 0 * 768 + fh * 128 + 128],
                        xT_c[c][:, 0:512], start=True, stop=False,
                    )
                    nc.tensor.matmul(
                        ps_q,
                        wa_b[:, 1 * 768 + fh * 128: 1 * 768 + fh * 128 + 128],
                        xT_c[c][:, 512:1024], start=False, stop=True,
                    )
                    nc.scalar.activation(
                        qTt[fh][c][:], ps_q, mybir.ActivationFunctionType.Copy,
                    )
                    ps_k = pset.tile([128, 512], F32, tag="mm", bufs=2)
                    nc.tensor.matmul(
                        ps_k,
                        wa_b[:, 0 * 768 + 256 + fh * 128:
                             0 * 768 + 256 + fh * 128 + 128],
                        xT_c[c][:, 0:512], start=True, stop=False,
                    )
                    nc.tensor.matmul(
                        ps_k,
                        wa_b[:, 1 * 768 + 256 + fh * 128:
                             1 * 768 + 256 + fh * 128 + 128],
                        xT_c[c][:, 512:1024], start=False, stop=True,
                    )
                    nc.scalar.activation(
                        kTt[fh][c][:], ps_k, mybir.ActivationFunctionType.Copy,
                    )
                # v for the 4 token tiles of this chunk
                for nl in range(4):
                    ps_v = pset.tile([128, 256], F32, tag="mm", bufs=2)
                    for kc in range(2):
                        nc.tensor.matmul(
                            ps_v,
                            xT_c[c][:, kc * 512 + nl * 128:
                                    kc * 512 + nl * 128 + 128],
                            wa_b[:, kc * 768 + 512: kc * 768 + 768],
                            start=(kc == 0),
                            stop=(kc == 1),
                        )
                    nc.vector.tensor_copy(
                        v65c[c][:, nl * 260: nl * 260 + 260].rearrange(
                            "p (g c2) -> p g c2", g=4)[:, :, 0:64],
                        ps_v.rearrange("p (g c2) -> p g c2", g=4),
                    )

        # ---- attention: software-pipelined scores/exp/PV ----
        with tc.tile_pool(name="pat", bufs=1, space="PSUM") as pat:
            items = []
            for hp in range(2):          # head pair: global heads (2hp, 2hp+1)
                for tqb in range(NQB):
                    ntk = 4 * (tqb + 1)
                    tiles = [(h, tk) for tk in range(ntk) for h in range(2)]
                    groups = [
                        tiles[i: i + ATT_GROUP]
                        for i in range(0, len(tiles), ATT_GROUP)
                    ]
                    for gi, grp in enumerate(groups):
                        items.append({
                            "hp": hp, "tqb": tqb, "grp": grp, "ntk": ntk,
                            "first": gi == 0, "last": gi == len(groups) - 1,
                        })

            def emit_scores_exp(it):
                hp, tqb, grp = it["hp"], it["tqb"], it["grp"]
                gw = 512 * len(grp)
                sg = pat.tile([128, 512 * ATT_GROUP], F32, tag="sg", bufs=2)
                pg = sb.tile([128, 512 * ATT_GROUP], BF16, tag="P", bufs=6,
                             name="pg")
                for j, (h, tk) in enumerate(grp):
                    nc.tensor.matmul(
                        sg[:, j * 512:(j + 1) * 512],
                        kTt[hp][tk // 4][64 * h: 64 * h + 64,
                                         (tk % 4) * 128:(tk % 4) * 128 + 128],
                        qTt[hp][tqb][64 * h: 64 * h + 64, :],
                        start=True, stop=True,
                    )
                # P = 2^(S^T)  (scores already in log2 units)
                nc.scalar.activation(
                    pg[:, :gw], sg[:, :gw],
                    mybir.ActivationFunctionType.Exp, scale=LN2,
                )
                h0, tk0 = grp[0]
                if tk0 >= 4 * tqb:  # diagonal tiles: zero the triangle in
                    # both heads' tiles with one select (outer stride-0
                    # pattern repeats the same triangle 512 cols apart);
                    # cols below off are skipped by the off-sliced PV
                    off = (tk0 - 4 * tqb) * 128
                    dv = pg.rearrange("p (g c) -> p g c", g=2)[:, :, off: off + 128]
                    nc.gpsimd.affine_select(
                        out=dv, in_=dv,
                        compare_op=mybir.AluOpType.is_ge,
                        fill=0.0,
                        base=0,
                        pattern=[[0, 2], [1, 128]],
                        channel_multiplier=-1,
                    )
                it["pg"] = pg

            def emit_pv(it, acc):
                hp, tqb, ntk = it["hp"], it["tqb"], it["ntk"]
                pg = it["pg"]
                for j, (h, tk) in enumerate(it["grp"]):
                    gh = 2 * hp + h
                    off = (tk - 4 * tqb) * 128 if tk >= 4 * tqb else 0
                    nc.tensor.matmul(
                        acc[h][0:65, off:],
                        v65c[tk // 4][:, (tk % 4) * 260 + gh * 65:
                                      (tk % 4) * 260 + gh * 65 + 65],
                        pg[:, j * 512 + off:(j + 1) * 512],
                        start=(tk == 0), stop=(tk == ntk - 1),
                    )

            def emit_normalize(acc, hp, tqb):
                """yT = O^T / rowsums. Stage O^T rows and the sums row to
                SBUF (frees the PSUM banks; PSUM-direct multiplies slow the
                PE/ACT streams via port contention), broadcast the sums,
                one fast-approx reciprocal, two multiplies."""
                yt = yTt[hp][tqb]
                oc = sb.tile([128, 1024], F32, tag="ocopy", bufs=2, name="oc")
                nc.vector.tensor_copy(oc[0:65, 0:512], acc[0][0:65, :])
                nc.vector.tensor_copy(oc[0:65, 512:1024], acc[1][0:65, :])
                srow = sb.tile([1, 1024], F32, tag="srow", bufs=2, name="srow")
                nc.vector.tensor_copy(srow[0:1, :], oc[64:65, :])
                sr = sb.tile([128, 1024], F32, tag="bcast", bufs=2, name="sr")
                nc.gpsimd.partition_broadcast(sr[0:64, :], srow[0:1, :],
                                              channels=64)
                rb = sb.tile([128, 1024], F32, tag="recip", bufs=2, name="rb")
                nc.vector.reciprocal_approx_fast(rb[0:64, :], sr[0:64, :])
                nc.vector.tensor_mul(yt[0:64, :], oc[0:64, 0:512],
                                     rb[0:64, 0:512])
                nc.vector.tensor_mul(
                    yt[64:128, :], oc[0:64, 512:1024], rb[0:64, 512:1024]
                )

            prev = None
            acc = None
            for it in items + [None]:
                if it is not None:
                    emit_scores_exp(it)
                if prev is not None:
                    if prev["first"]:
                        oa = pat.tile([128, 512], F32, tag="oacc", bufs=4,
                                      name="oa")
                        ob = pat.tile([128, 512], F32, tag="oacc", bufs=4,
                                      name="ob")
                        acc = (oa, ob)
                    emit_pv(prev, acc)
                    if prev["last"]:
                        emit_normalize(acc, prev["hp"], prev["tqb"])
                prev = it

        # ---- output projection (deep-buffered tail) ----
        with tc.tile_pool(name="ppr", bufs=1, space="PSUM") as ppr:
            for tqb in range(NQB):
                for half in range(2):
                    psz = ppr.tile([128, 512], F32, tag="pz", bufs=4)
                    for sub in range(2):
                        nloc = half * 2 + sub          # token tile within block
                        for fh in range(2):
                            nc.tensor.matmul(
                                psz[:, sub * 256:(sub + 1) * 256],
                                yTt[fh][tqb][:, nloc * 128:(nloc + 1) * 128],
                                wp_b[:, fh * 256: fh * 256 + 256],
                                start=(fh == 0),
                                stop=(fh == 1),
                            )
                    z_sb = sb.tile([128, 512], F32, tag="z", bufs=8, name="z_sb")
                    nc.vector.tensor_copy(z_sb, psz)
                    n0 = tqb * 4 + half * 2
                    nc.sync.dma_start(
                        y_d[:].rearrange("(n p) c -> p n c", p=128)[:, n0: n0 + 2],
                        z_sb.rearrange("p (n c) -> p n c", n=2),
                    )
        sb.release()
    nc.compile()
    return nc


def _get_nc():
    global _cached_nc
    if _cached_nc is None:
        _cached_nc = _build()
    return _cached_nc


def kernel(**inputs):
    from concourse.bass_utils import run_bass_kernel_spmd

    x = np.ascontiguousarray(np.asarray(inputs["x"], dtype=np.float32))
    wa = np.ascontiguousarray(np.asarray(inputs["W_attn"], dtype=np.float32))
    wp = np.ascontiguousarray(np.asarray(inputs["W_proj"], dtype=np.float32))
    nc = _get_nc()
    in_maps = [
        {"x": np.ascontiguousarray(x[b]), "W_attn": wa, "W_proj": wp}
        for b in range(B)
    ]
    res = run_bass_kernel_spmd(nc, in_maps, core_ids=list(range(B)))
    return np.stack([res.results[b]["y"] for b in range(B)], axis=0)


# revision 20
# speedup vs baseline: 1.1670x; 1.1670x over previous
"""Causal self-attention Trainium2 kernel (B=8, T=2048, C=256, H=4).

Sharding: batch B=8 across the 8 NeuronCores (data parallel, no collectives).
Each core computes one batch element end-to-end:
  qkv = x @ W_attn ; per-head causal softmax(q k^T / sqrt(hs)) @ v ; @ W_proj

Layout strategy (per core):
  - x streamed in 4 token chunks of 512: per chunk DMA -> bf16 cast (DVE)
    -> PE transpose -> q/k matmuls (drained on ACT; Copy shares the exp
    act table) -> v matmuls (drained on DVE). Per-chunk tiles keep the
    dependencies fine-grained so the PE starts as soon as chunk 0 lands.
  - qT,kT computed transposed (feature rows on partitions); the softmax
    scale * log2(e) is folded into the W_attn q-columns at the bf16 cast
    so scores come out of the PE in log2 units.
  - S^T tiles (k on partitions, q on free dim) = kT_tile.T @ qT_block; the
    two heads of a pair are emitted back-to-back with K=64 row groups 0/64
    so they pack concurrently in the PE array (ATT_GROUP=2 keeps every
    pair emission-adjacent).
  - exp2 via ScalarE activation(Exp, scale=ln2) over 2-bank PSUM groups.
  - causal mask on diagonal 128x128 blocks via gpsimd affine_select on P.
  - O^T += V_tile.T @ P (per-head accumulators, M=65: the 65th stationary
    column is ones so row 64 of O^T accumulates the softmax row sums).
  - Software pipelining: emission order scores(g), scores(g+1), PV(g), ...
    so the in-order PE never waits on the ACT exp of the current group;
    oacc bufs=4 so a new round's PV never waits on the previous round's
    normalization reads.
  - normalization: O^T staged to SBUF (PSUM-direct reads measurably slow
    the PE/ACT streams via PSUM port contention), sums broadcast from
    partition 0, custom-DVE reciprocal_approx_fast, two multiplies into
    per-round yT tiles.
  - proj tail: z = Y @ W_proj, deep-buffered (psz bufs=4, z bufs=8) so the
    output DMA latency never stalls the matmul stream.
"""

import sys

if "/opt/trn_rl_repo" not in sys.path:
    sys.path.insert(0, "/opt/trn_rl_repo")

import numpy as np

import concourse.bass as bass
import concourse.mybir as mybir
from concourse import bacc
from concourse.masks import make_identity
from concourse.tile import TileContext

B, T, C = 8, 2048, 256
H, HS = 4, 64
NT = T // 128            # 16 token tiles
NQB = T // 512           # 4 q blocks of 512
F32 = mybir.dt.float32
BF16 = mybir.dt.bfloat16
LOG2E = 1.4426950408889634
LN2 = 0.6931471805599453
QSCALE = LOG2E / 8.0     # softmax scale 1/sqrt(hs) in log2 units
ATT_GROUP = 2            # S tiles per exp2 activation (one head pair)

_cached_nc = None


def _build():
    nc = bacc.Bacc("TRN2", target_bir_lowering=False, debug=False)
    x_d = nc.declare_dram_parameter("x", [T, C], F32, isOutput=False)
    wa_d = nc.declare_dram_parameter("W_attn", [C, 3 * C], F32, isOutput=False)
    wp_d = nc.declare_dram_parameter("W_proj", [C, C], F32, isOutput=False)
    y_d = nc.declare_dram_parameter("y", [T, C], F32, isOutput=True)

    with TileContext(nc) as tc:
        sb = tc.alloc_tile_pool(name="sb", bufs=1)
        # per-chunk input tiles (chunk = 512 tokens = 4 token tiles)
        x_c = [sb.tile([128, 1024], F32, name=f"x{c}") for c in range(4)]
        xb_c = [sb.tile([128, 1024], BF16, name=f"xb{c}") for c in range(4)]
        # xT chunk: [c_in 128, (kc, t512)]
        xT_c = [sb.tile([128, 1024], BF16, name=f"xT{c}") for c in range(4)]
        # qT/kT per (fh, nb): [feat 128, t512]; feat rows 0-63 = head 2fh,
        # 64-127 = head 2fh+1
        qTt = [[sb.tile([128, 512], BF16, name=f"qT{fh}_{nb}")
                for nb in range(NQB)] for fh in range(2)]
        kTt = [[sb.tile([128, 512], BF16, name=f"kT{fh}_{nb}")
                for nb in range(NQB)] for fh in range(2)]
        # v with a ones column per head: [t128, (n4, gh, 65)]; col 64 == 1.0
        v65c = [sb.tile([128, 4 * 260], BF16, name=f"v65_{c}")
                for c in range(4)]
        # yT split per (hp, tqb)
        yTt = [[sb.tile([128, 512], BF16, name=f"yT{hp}_{tqb}")
                for tqb in range(NQB)] for hp in range(2)]
        wa_f = sb.tile([128, 2 * 768], F32, name="wa_f")
        wa_b = sb.tile([128, 2 * 768], BF16, name="wa_b")
        wp_f = sb.tile([128, 2 * 256], F32, name="wp_f")
        wp_b = sb.tile([128, 2 * 256], BF16, name="wp_b")
        ident = sb.tile([128, 128], F32, name="ident")
        identb = sb.tile([128, 128], BF16, name="identb")

        make_identity(nc, ident)
        nc.vector.tensor_copy(identb, ident)
        for c in range(4):
            nc.gpsimd.memset(v65c[c], 1.0)  # ones cols survive the v copies

        # ---- load inputs (x in 4 chunks so the pipeline starts early) ----
        nc.sync.dma_start(
            wa_f.rearrange("p (k m) -> p k m", k=2),
            wa_d[:].rearrange("(k p) m -> p k m", p=128),
        )
        nc.sync.dma_start(
            wp_f.rearrange("p (k m) -> p k m", k=2),
            wp_d[:].rearrange("(k p) m -> p k m", p=128),
        )
        for c in range(4):
            nc.sync.dma_start(
                x_c[c].rearrange("p (n c2) -> p n c2", n=4),
                x_d[c * 512:(c + 1) * 512].rearrange(
                    "(n p) c2 -> p n c2", p=128),
            )
        # DVE emission order matters: the first PE transposes wait on the
        # chunk-0 cast, so it goes first; wp_b is only read by the proj
        # tail, so its cast goes last
        nc.vector.tensor_copy(xb_c[0], x_c[0])
        # bf16 weight cast; QSCALE folded into the q columns of W_attn
        for kc in range(2):
            nc.vector.tensor_scalar_mul(
                wa_b[:, kc * 768: kc * 768 + 256],
                wa_f[:, kc * 768: kc * 768 + 256],
                QSCALE,
            )
            nc.vector.tensor_copy(
                wa_b[:, kc * 768 + 256: kc * 768 + 768],
                wa_f[:, kc * 768 + 256: kc * 768 + 768],
            )
        for c in range(1, 4):
            nc.vector.tensor_copy(xb_c[c], x_c[c])
        nc.vector.tensor_copy(wp_b[:], wp_f[:])

        # ---- streamed setup: per chunk cast -> transpose -> qkv ----
        with tc.tile_pool(name="pset", bufs=1, space="PSUM") as pset:
            for c in range(4):
                for kc in range(2):
                    tp = pset.tile([128, 512], BF16, tag="tp", bufs=2)
                    for j in range(4):
                        nc.tensor.transpose(
                            tp[:, j * 128:(j + 1) * 128],
                            xb_c[c][:, j * 256 + kc * 128:
                                    j * 256 + kc * 128 + 128],
                            identb,
                        )
                    nc.vector.tensor_copy(
                        xT_c[c][:, kc * 512:(kc + 1) * 512], tp[:]
                    )
                # q/k for this token block (nb == c), both feature halves
                for fh in range(2):
                    ps_q = pset.tile([128, 512], F32, tag="mm", bufs=2)
                    nc.tensor.matmul(
                        ps_q,
                        wa_b[:, 0 * 768 + fh * 128: 0 * 768 + fh * 128 + 128],
                        xT_c[c][:, 0:512], start=True, stop=False,
                    )
                    nc.tensor.matmul(
                        ps_q,
                        wa_b[:, 1 * 768 + fh * 128: 1 * 768 + fh * 128 + 128],
                        xT_c[c][:, 512:1024], start=False, stop=True,
                    )
                    nc.scalar.activation(
                        qTt[fh][c][:], ps_q, mybir.ActivationFunctionType.Copy,
                    )
                    ps_k = pset.tile([128, 512], F32, tag="mm", bufs=2)
                    nc.tensor.matmul(
                        ps_k,
                        wa_b[:, 0 * 768 + 256 + fh * 128:
                             0 * 768 + 256 + fh * 128 + 128],
                        xT_c[c][:, 0:512], start=True, stop=False,
                    )
                    nc.tensor.matmul(
                        ps_k,
                        wa_b[:, 1 * 768 + 256 + fh * 128:
                             1 * 768 + 256 + fh * 128 + 128],
                        xT_c[c][:, 512:1024], start=False, stop=True,
                    )
                    nc.scalar.activation(
                        kTt[fh][c][:], ps_k, mybir.ActivationFunctionType.Copy,
                    )
                # v for the 4 token tiles of this chunk
                for nl in range(4):
                    ps_v = pset.tile([128, 256], F32, tag="mm", bufs=2)
                    for kc in range(2):
                        nc.tensor.matmul(
                            ps_v,
                            xT_c[c][:, kc * 512 + nl * 128:
                                    kc * 512 + nl * 128 + 128],
                            wa_b[:, kc * 768 + 512: kc * 768 + 768],
                            start=(kc == 0),
                            stop=(kc == 1),
                        )
                    nc.vector.tensor_copy(
                        v65c[c][:, nl * 260: nl * 260 + 260].rearrange(
                            "p (g c2) -> p g c2", g=4)[:, :, 0:64],
                        ps_v.rearrange("p (g c2) -> p g c2", g=4),
                    )

        # ---- attention: software-pipelined scores/exp/PV ----
        with tc.tile_pool(name="pat", bufs=1, space="PSUM") as pat:
            items = []
            for hp in range(2):          # head pair: global heads (2hp, 2hp+1)
                for tqb in range(NQB):
                    ntk = 4 * (tqb + 1)
                    tiles = [(h, tk) for tk in range(ntk) for h in range(2)]
                    groups = [
                        tiles[i: i + ATT_GROUP]
                        for i in range(0, len(tiles), ATT_GROUP)
                    ]
                    for gi, grp in enumerate(groups):
                        items.append({
                            "hp": hp, "tqb": tqb, "grp": grp, "ntk": ntk,
                            "first": gi == 0, "last": gi == len(groups) - 1,
                        })

            def emit_scores_exp(it):
                hp, tqb, grp = it["hp"], it["tqb"], it["grp"]
                gw = 512 * len(grp)
                sg = pat.tile([128, 512 * ATT_GROUP], F32, tag="sg", bufs=2)
                pg = sb.tile([128, 512 * ATT_GROUP], BF16, tag="P", bufs=6,
                             name="pg")
                for j, (h, tk) in enumerate(grp):
                    nc.tensor.matmul(
                        sg[:, j * 512:(j + 1) * 512],
                        kTt[hp][tk // 4][64 * h: 64 * h + 64,
                                         (tk % 4) * 128:(tk % 4) * 128 + 128],
                        qTt[hp][tqb][64 * h: 64 * h + 64, :],
                        start=True, stop=True,
                    )
                # P = 2^(S^T)  (scores already in log2 units)
                nc.scalar.activation(
                    pg[:, :gw], sg[:, :gw],
                    mybir.ActivationFunctionType.Exp, scale=LN2,
                )
                h0, tk0 = grp[0]
                if tk0 >= 4 * tqb:  # diagonal tiles: zero the triangle in
                    # both heads' tiles with one select (outer stride-0
                    # pattern repeats the same triangle 512 cols apart);
                    # cols below off are skipped by the off-sliced PV
                    off = (tk0 - 4 * tqb) * 128
                    dv = pg.rearrange("p (g c) -> p g c", g=2)[:, :, off: off + 128]
                    nc.gpsimd.affine_select(
                        out=dv, in_=dv,
                        compare_op=mybir.AluOpType.is_ge,
                        fill=0.0,
                        base=0,
                        pattern=[[0, 2], [1, 128]],
                        channel_multiplier=-1,
                    )
                it["pg"] = pg

            def emit_pv(it, acc):
                hp, tqb, ntk = it["hp"], it["tqb"], it["ntk"]
                pg = it["pg"]
                for j, (h, tk) in enumerate(it["grp"]):
                    gh = 2 * hp + h
                    off = (tk - 4 * tqb) * 128 if tk >= 4 * tqb else 0
                    nc.tensor.matmul(
                        acc[h][0:65, off:],
                        v65c[tk // 4][:, (tk % 4) * 260 + gh * 65:
                                      (tk % 4) * 260 + gh * 65 + 65],
                        pg[:, j * 512 + off:(j + 1) * 512],
                        start=(tk == 0), stop=(tk == ntk - 1),
                    )

            def emit_normalize(acc, hp, tqb):
                """yT = O^T / rowsums. Stage O^T rows and the sums row to
                SBUF (frees the PSUM banks; PSUM-direct multiplies slow the
                PE/ACT streams via port contention), broadcast the sums,
                one fast-approx reciprocal, two multiplies."""
                yt = yTt[hp][tqb]
                oc = sb.tile([128, 1024], F32, tag="ocopy", bufs=2, name="oc")
                nc.vector.tensor_copy(oc[0:65, 0:512], acc[0][0:65, :])
                nc.vector.tensor_copy(oc[0:65, 512:1024], acc[1][0:65, :])
                srow = sb.tile([1, 1024], F32, tag="srow", bufs=2, name="srow")
                nc.vector.tensor_copy(srow[0:1, :], oc[64:65, :])
                sr = sb.tile([128, 1024], F32, tag="bcast", bufs=2, name="sr")
                nc.gpsimd.partition_broadcast(sr[0:64, :], srow[0:1, :],
                                              channels=64)
                rb = sb.tile([128, 1024], F32, tag="recip", bufs=2, name="rb")
                nc.vector.reciprocal_approx_fast(rb[0:64, :], sr[0:64, :])
                nc.vector.tensor_mul(yt[0:64, :], oc[0:64, 0:512],
                                     rb[0:64, 0:512])
                nc.vector.tensor_mul(
                    yt[64:128, :], oc[0:64, 512:1024], rb[0:64, 512:1024]
                )

            prev = None
            acc = None
            for it in items + [None]:
                if it is not None:
                    emit_scores_exp(it)
                if prev is not None:
                    if prev["first"]:
                        oa = pat.tile([128, 512], F32, tag="oacc", bufs=4,
                                      name="oa")
                        ob = pat.tile([128, 512], F32, tag="oacc", bufs=4,
                                      name="ob")
                        acc = (oa, ob)
                    emit_pv(prev, acc)
                    if prev["last"]:
                        emit_normalize(acc, prev["hp"], prev["tqb"])
                prev = it

        # ---- output projection (deep-buffered tail) ----
        with tc.tile_pool(name="ppr", bufs=1, space="PSUM") as ppr:
            for tqb in range(NQB):
                for half in range(2):
                    psz = ppr.tile([128, 512], F32, tag="pz", bufs=4)
                    for sub in range(2):
                        nloc = half * 2 + sub          # token tile within block
                        for fh in range(2):
                            nc.tensor.matmul(
                                psz[:, sub * 256:(sub + 1) * 256],
                                yTt[fh][tqb][:, nloc * 128:(nloc + 1) * 128],
                                wp_b[:, fh * 256: fh * 256 + 256],
                                start=(fh == 0),
                                stop=(fh == 1),
                            )
                    z_sb = sb.tile([128, 512], F32, tag="z", bufs=8, name="z_sb")
                    nc.vector.tensor_copy(z_sb, psz)
                    n0 = tqb * 4 + half * 2
                    nc.sync.dma_start(
                        y_d[:].rearrange("(n p) c -> p n c", p=128)[:, n0: n0 + 2],
                        z_sb.rearrange("p (n c) -> p n c", n=2),
                    )
        sb.release()
    nc.compile()
    return nc


def _get_nc():
    global _cached_nc
    if _cached_nc is None:
        _cached_nc = _build()
    return _cached_nc


def kernel(**inputs):
    from concourse.bass_utils import run_bass_kernel_spmd

    x = np.ascontiguousarray(np.asarray(inputs["x"], dtype=np.float32))
    wa = np.ascontiguousarray(np.asarray(inputs["W_attn"], dtype=np.float32))
    wp = np.ascontiguousarray(np.asarray(inputs["W_proj"], dtype=np.float32))
    nc = _get_nc()
    in_maps = [
        {"x": np.ascontiguousarray(x[b]), "W_attn": wa, "W_proj": wp}
        for b in range(B)
    ]
    res = run_bass_kernel_spmd(nc, in_maps, core_ids=list(range(B)))
    return np.stack([res.results[b]["y"] for b in range(B)], axis=0)
